# revision 70
# baseline (speedup 1.0000x reference)
"""Trainium2 Bass kernel: pre-LN transformer block (B=4, T=2048, E=1024, H=16, FFN=100).

Sharding (8 NeuronCores): core 2b+g handles batch b, head-group g (8 of 16 heads,
i.e. a 512-wide slice of the QKV output dim / proj input dim).  Both cores of a
pair compute attention + proj partials for all 2048 tokens of their batch; two
per-pair bf16 ReduceScatters combine the partials and hand each core half the
tokens, on which it runs LN2 + FFN and writes its [1024, 1024] output shard.

Schedule: LN1+QKV chunks 0-2, then attention chunks start interleaving -
att(0), QKV(3), att(2), att(1), att(3) - with proj subtiles of each completed
chunk (and the first FFN region) emitted as PE filler inside the next
attention chunk's head-pair loop.  The first ReduceScatter overlaps att(3);
only the second one plus the last FFN region form the serial tail.  exp is
issued once per t_k tile covering both heads of the pair ([128, 2, 512] PSUM
tile spanning two banks), which amortizes the Act engine's fixed per-
instruction cost and keeps the attention inner loop fed fast enough that the
PE stays at full clock.

SPMD notes: all 8 cores run one program; per-core behavior differs only via
input data.  The residual is fed as x/2 on both pair members (summed back to x
by the reduce); LN1 uses eps/4 so layernorm(x/2, eps/4) == layernorm(x, eps)
exactly.  b_proj/2 is folded host-side into the proj residual copy of x (xb),
and b2 is folded into the FFN second matmul as an extra input row.

Attention layout: scores are computed transposed, S^T[t_k, t_q] = k^T.T @ q^T,
with q^T/k^T in [head_dim, token] layout (from PE-transposed LN output, all in
bf16 so transposes run at 1 cycle/row).  Softmax runs without max subtraction
(logits are ~N(0, 0.25), safe in fp32): exp on ScalarE straight out of PSUM
with the 1/sqrt(E) scale folded in.  Causality is exploited at 128-column
granularity: for the diagonal t_k tile at offset m only the suffix columns
[m*128:) are computed/exp'd, and a single [128,128] lower-triangle mask fixes
the diagonal block.  The denominator comes from an extra ones-column appended
to V; its reciprocal row is broadcast across partitions 64:128 of the same AV
PSUM bank via a K=1 ones matmul (fully on-chip - no DRAM bounce), and one DVE
multiply produces the normalized attention output.
"""

from contextlib import ExitStack

import numpy as np
import ml_dtypes

import concourse.bass as bass
import concourse.mybir as mybir
import concourse.tile as tile
from concourse.bass_utils import run_bass_kernel_spmd
from concourse.vector_clock import ScopedClock


class SplitDrainTC(tile.TileContext):
    """Works around a walrus codegen limit: an SP CTRL instruction may carry
    only one sync wait, so the kernel-tail drain's waits are split onto
    preceding single-wait nops."""

    def _drain_and_barrier(self, tick_clock, wait_clock):
        probe = self.nc.sync.nop(nofuse=True)
        wait_clock.add_sem_waits(
            probe.ins, ScopedClock({None: tick_clock.global_clock})
        )
        si = probe.ins.sync_info
        waits = list(si.on_wait) if si is not None else []
        if len(waits) > 1:
            si.on_wait = [waits[0]]
            for w in waits[1:]:
                n2 = self.nc.sync.nop(nofuse=True)
                n2.ins.sync_info = mybir.SyncInfo(on_wait=[w], on_update=[])
        self.nc.sync.drain()
        self.nc.all_engine_barrier()
        popped = self.nc._tile_sem_poison_stack.pop()
        assert popped is self._sem_poison
        self.nc.clear_and_free_semaphores(list(self.sems.allocated().values()))
        self.nc.all_engine_barrier()

B, T, E, H, HS, FFN = 4, 2048, 1024, 16, 64, 100
EPS = 1e-5
NCORE = 8
TC = 512            # token chunk
NTC = T // TC       # 4
TS = 128            # token subtile
NSUB = TC // TS     # 4
ET = 128            # embed tile
NET = E // ET       # 8
DSL = E // 2        # per-core qkv output slice (8 heads * 64)
NDT = DSL // 128    # 4 d-tiles (2 heads each)
HPC = H // 2        # 8 heads per core
SCALE = float(E) ** -0.5
PAIRS = [[0, 1], [2, 3], [4, 5], [6, 7]]

MM_MODE = "bf16"    # "bf16" | "f32"
AF = mybir.ActivationFunctionType


def _mdt(mode):
    return mybir.dt.bfloat16 if mode == "bf16" else mybir.dt.float32


def _np_mdt(mode):
    return ml_dtypes.bfloat16 if mode == "bf16" else np.float32


def build(mode=MM_MODE):
    f32 = mybir.dt.float32
    mdt = _mdt(mode)

    nc = bass.Bass(num_devices=NCORE)

    io = {}

    def param(name, shape, dtype):
        io[name] = nc.declare_dram_parameter(name, shape, dtype, isOutput=False)

    param("xr", [T, E], mdt)           # x/2 (LN1 input)
    param("xb", [T, E], mdt)           # x/2 + b_proj/2 (proj residual)
    # weights come in pre-rearranged host-side ([partition, k, d]) so each
    # load is one contiguous 8KB-per-partition DMA instead of 1K row reads
    param("wq", [ET, NET, DSL], mdt)
    param("wk", [ET, NET, DSL], mdt)
    param("wv", [ET, NET, DSL], mdt)
    param("wp", [128, NDT, E], mdt)
    param("w1", [ET, NET, FFN], mdt)
    param("w2e", [FFN + 1, E], mdt)    # w2 with b2 as the extra last row
    param("b1", [FFN, 1], f32)
    param("ln1g", [E, 1], f32)
    param("ln1b", [E, 1], f32)
    param("ln2g", [E, 1], f32)
    param("ln2b", [E, 1], f32)
    param("mask", [TS, 2, TS], mdt)    # lower-triangle block mask, both heads
    param("ident", [TS, TS], mdt)
    io["out"] = nc.declare_dram_parameter("out", [T // 2, E], f32, isOutput=True)

    with SplitDrainTC(nc) as tc:
        with ExitStack() as ctx:
            _build_tile(ctx, tc, nc, mode, mdt, f32, io)
    _split_waits(nc)
    return nc


def _split_waits(nc, maxw=1):
    """walrus codegen accepts a limited number of sync waits per instruction;
    move the excess onto same-engine NoOps inserted just before."""
    import bass_rust
    n = 0
    for f in nc.m.functions:
        for b in f.blocks:
            new = []
            for inst in b.instructions:
                si = inst.sync_info
                # fixed-length ISA instructions can't carry waits at all
                cap = 0 if isinstance(inst, bass_rust.InstISA) else maxw
                if si is not None and len(si.on_wait) > cap:
                    waits = list(si.on_wait)
                    keep = waits[-cap:] if cap else []
                    excess = waits[:-cap] if cap else waits
                    for w in excess:
                        nop = mybir.InstNoOp(
                            name=f"{inst.name}-wsplit{n}", engine=inst.engine
                        )
                        nop.bass_nofuse = True
                        n += 1
                        nop.sync_info = mybir.SyncInfo(
                            on_wait=[w], on_update=[]
                        )
                        new.append(nop)
                    si.on_wait = keep
                new.append(inst)
            if n:
                b.instructions = new


def _build_tile(ctx, tc, nc, mode, mdt, f32, io):
    xr, xb, out = io["xr"], io["xb"], io["out"]

    def pool(name, bufs, space="SBUF"):
        return ctx.enter_context(tc.tile_pool(name=name, bufs=bufs, space=space))

    # ---- internal DRAM ----
    dram = pool("dram", 1, space="DRAM")
    ar_in = dram.tile([T, E], mdt, name="ar_in")
    rs_out0 = dram.tile([TC, E], mdt, name="rs0_out")
    # region B is reduced as two half-size collectives so the last one
    # overlaps the first half of ffn_region(1)
    rs_outB = [
        dram.tile([2 * TS, E], mdt, name="rsb0_out"),
        dram.tile([2 * TS, E], mdt, name="rsb1_out"),
    ]

    # ---- persistent SBUF: weights & constants ----
    wpool = pool("weights", 1)
    wq_sb = wpool.tile([ET, NET, DSL], mdt, name="wq_sb")
    wk_sb = wpool.tile([ET, NET, DSL], mdt, name="wk_sb")
    wv_sb = wpool.tile([ET, NET, DSL], mdt, name="wv_sb")
    def load_qkv_weights():
        # issued after chunk-0's x loads so the first LN tiles win the DMA
        # engines; the SP queue config is instant and QKV matmuls only need
        # these at ~35us
        nc.sync.dma_start(out=wq_sb, in_=io["wq"][:])
        nc.sync.dma_start(out=wk_sb, in_=io["wk"][:])
        nc.sync.dma_start(out=wv_sb, in_=io["wv"][:])

    wp_sb = wpool.tile([128, NDT, E], mdt, name="wp_sb")
    ones64b = wpool.tile([1, HS], mdt, name="ones64b")
    nc.vector.memset(ones64b, 1.0)
    ones65 = wpool.tile([HS + 1, 1], f32, name="ones65")
    nc.vector.memset(ones65, 1.0)
    w1_sb = wpool.tile([ET, NET, FFN], mdt, name="w1_sb")
    w2_sb = wpool.tile([FFN + 1, E], mdt, name="w2_sb")
    b1_sb = wpool.tile([FFN, 1], f32, name="b1_sb")

    def load_late_weights():
        # proj/FFN weights are not needed until well into the kernel; loading
        # them here keeps their DMA descriptors out of the startup queues
        nc.sync.dma_start(out=wp_sb, in_=io["wp"][:])
        nc.sync.dma_start(out=w1_sb, in_=io["w1"][:])
        nc.sync.dma_start(out=w2_sb, in_=io["w2e"][:])
        nc.sync.dma_start(out=b1_sb, in_=io["b1"][:])
        nc.sync.dma_start(out=mask_sb, in_=io["mask"][:])
        for nm in ("ln2g", "ln2b"):
            nc.sync.dma_start(
                out=ln_sb[nm], in_=io[nm].rearrange("(k p) o -> p k o", p=ET)
            )
    ln_sb = {}
    for nm in ("ln1g", "ln1b", "ln2g", "ln2b"):
        ln_sb[nm] = wpool.tile([ET, NET, 1], f32, name=nm + "_sb")
    for nm in ("ln1g", "ln1b"):
        nc.gpsimd.dma_start(
            out=ln_sb[nm], in_=io[nm].rearrange("(k p) o -> p k o", p=ET)
        )
    mask_sb = wpool.tile([TS, 2, TS], mdt, name="mask_sb")
    id_sb = wpool.tile([TS, TS], mdt, name="id_sb")
    nc.gpsimd.dma_start(out=id_sb, in_=io["ident"][:])
    eps1_sb = wpool.tile([128, 1], f32, name="eps1_sb")
    nc.vector.memset(eps1_sb, EPS / 4.0)  # LN1 runs on x/2
    eps2_sb = wpool.tile([128, 1], f32, name="eps2_sb")
    nc.vector.memset(eps2_sb, EPS)

    # ---- persistent SBUF: per-chunk K^T, V(+ones), Q^T ----
    kv = pool("kv", 1)
    kT_c = [kv.tile([128, NDT, TC], mdt, name=f"kT{c}") for c in range(NTC)]
    vt_c = [kv.tile([128, NSUB, HPC, HS + 1], mdt, name=f"vt{c}")
            for c in range(NTC)]
    qT_c = [kv.tile([128, NDT, TC], mdt, name=f"qT{c}") for c in range(NTC)]

    # ---- working pools ----
    xt_pool = pool("xt", 4)
    h_pool = pool("h", 5)
    mv_pool = pool("mv", 3)
    hT_pool = pool("hT", 2)
    pt_pool = pool("pt", 6)
    avs_pool = pool("avs", 4)
    rec_pool = pool("rec", 4)
    attT_pool = pool("attT", 8)   # att(0) and att(2) outputs both live
    xb_pool = pool("xbp", 2)
    part_pool = pool("part", 3)
    x2_pool = pool("x2", 4)
    f1_pool = pool("f1", 2)
    out_pool = pool("outp", 2)
    ps_mm = pool("ps_mm", 2, space="PSUM")
    ps_sc = pool("ps_sc", 2, space="PSUM")
    ps_av = pool("ps_av", 2, space="PSUM")

    def layer_norm_chunk(x_ts, eps_tile, out_ts):
        """out_ts[s] (bf16) = (x_ts[s] - mean) * rsqrt(var + eps), with the
        4 subtiles' stats batched so Sqrt costs one Act instruction (one
        activation-table region instead of four)."""
        n = len(x_ts)
        mv = mv_pool.tile([128, n, 2], f32, name="mv")
        for s, x_t in enumerate(x_ts):
            stats = mv_pool.tile(
                [128, 2, nc.vector.BN_STATS_DIM], f32, name="stats"
            )
            xg = x_t.rearrange("p (u q) -> p u q", u=2)
            for u in range(2):
                nc.vector.bn_stats(out=stats[:, u, :], in_=xg[:, u, :])
            nc.vector.bn_aggr(out=mv[:, s, :], in_=stats)
        rstd = mv_pool.tile([128, n], f32, name="rstd")
        nc.scalar.activation(
            out=rstd, in_=mv[:, :, 1], func=AF.Sqrt, bias=eps_tile, scale=1.0
        )
        nc.vector.reciprocal(out=rstd, in_=rstd)
        for s, x_t in enumerate(x_ts):
            nc.vector.tensor_scalar(
                out=out_ts[s], in0=x_t, scalar1=mv[:, s, 0:1],
                scalar2=rstd[:, s:s + 1],
                op0=mybir.AluOpType.subtract, op1=mybir.AluOpType.mult,
            )

    def transpose_cast(h_ts, g_sb, b_sb, hT, col0=0):
        """PE-transpose subtiles of h [128, E] (bf16) into hT[:, k, col0:],
        batching the 128x128 transposes of one k-tile into one PSUM tile so
        the layernorm scale/bias fold costs one DVE op per k-tile."""
        w = len(h_ts) * TS
        for k in range(NET):
            tp = ps_mm.tile([TS, TC], mdt, name="tp", tag="mm")
            for s, h_t in enumerate(h_ts):
                nc.tensor.transpose(
                    tp[:, s * TS:(s + 1) * TS],
                    h_t[:, k * ET:(k + 1) * ET], id_sb,
                )
            nc.vector.tensor_scalar(
                out=hT[:, k, col0:col0 + w], in0=tp[:, 0:w],
                scalar1=g_sb[:, k, :], scalar2=b_sb[:, k, :],
                op0=mybir.AluOpType.mult, op1=mybir.AluOpType.add,
            )

    # =====================================================================
    # Phase 1: LN1 + transpose + QKV per chunk
    # =====================================================================
    def ln_qkv_chunk(c, after_loads=None):
        hT = hT_pool.tile([ET, NET, TC], mdt, name="hT")
        x_ts = []
        h_ts = []
        for s in range(NSUB):
            r0 = c * TC + s * TS
            x_t = xt_pool.tile([128, E], mdt, name="x_t")
            nc.gpsimd.dma_start(out=x_t, in_=xr[r0:r0 + TS, :])
            x_ts.append(x_t)
            h_ts.append(h_pool.tile([128, E], mdt, name="h_t"))
        if after_loads is not None:
            after_loads()
        layer_norm_chunk(x_ts, eps1_sb, h_ts)
        transpose_cast(h_ts, ln_sb["ln1g"], ln_sb["ln1b"], hT)
        for dd in range(NDT):
            for w_sb, dst in ((wq_sb, qT_c[c]), (wk_sb, kT_c[c])):
                ps = ps_mm.tile([128, TC], f32, name="ps_qk", tag="mm")
                for k in range(NET):
                    nc.tensor.matmul(
                        ps, w_sb[:, k, dd * 128:(dd + 1) * 128],
                        hT[:, k, :],
                        start=(k == 0), stop=(k == NET - 1),
                    )
                nc.scalar.copy(dst[:, dd, :], ps)
        for s in range(NSUB):
            ps = ps_mm.tile([128, DSL], f32, name="ps_v", tag="mm")
            for k in range(NET):
                nc.tensor.matmul(
                    ps, hT[:, k, s * TS:(s + 1) * TS], wv_sb[:, k, :],
                    start=(k == 0), stop=(k == NET - 1),
                )
            nc.scalar.copy(
                vt_c[c][:, s, :, 0:HS],
                ps.rearrange("p (h d) -> p h d", h=HPC),
            )
            nc.gpsimd.memset(vt_c[c][:, s, :, HS:HS + 1], 1.0)

    # =====================================================================
    # Phase 2: attention + proj partials
    # =====================================================================
    def attention_chunk(c, fillers=()):
        """fillers: emitted one per head-pair iteration - independent PE work
        (proj subtiles of an earlier chunk, the first FFN region) that keeps
        the Tensor engine dense while Act paces the exp pipeline."""
        fillers = list(fillers)
        nkt = (c + 1) * NSUB
        attTs = []
        for pr in range(NDT):  # head pair = d-tile
            av_ps = [ps_av.tile([128, TC], f32, name="avp") for _ in range(2)]

            def do_scores(i):
                m = i - c * NSUB
                w0 = m * TS if m > 0 else 0  # suffix start (diagonal tiles)
                sc = ps_sc.tile([TS, 2, TC], f32, name="sc")  # 2 PSUM banks
                for hh in range(2):
                    h0 = hh * HS
                    nc.tensor.matmul(
                        sc[:, hh, w0:TC],
                        kT_c[i // NSUB][h0:h0 + HS, pr,
                                        (i % NSUB) * TS:(i % NSUB + 1) * TS],
                        qT_c[c][h0:h0 + HS, pr, w0:TC],
                        start=True, stop=True,
                    )
                pt = pt_pool.tile([TS, 2, TC], mdt, name="pt")
                # one exp covers both heads' score tiles
                nc.scalar.activation(
                    out=pt[:, :, w0:TC], in_=sc[:, :, w0:TC], func=AF.Exp,
                    scale=SCALE,
                )
                if m >= 0:
                    # diagonal block: zero t_k > t_q inside [m*TS,(m+1)*TS),
                    # both heads in one DVE op
                    nc.vector.tensor_mul(
                        pt[:, :, m * TS:(m + 1) * TS],
                        pt[:, :, m * TS:(m + 1) * TS], mask_sb,
                    )
                return (pt, w0)

            def do_avs(i, ptw):
                pt, w0 = ptw
                for hh in range(2):
                    nc.tensor.matmul(
                        av_ps[hh][0:HS + 1, w0:TC],
                        vt_c[i // NSUB][:, i % NSUB, pr * 2 + hh, :],
                        pt[:, hh, w0:TC],
                        start=(i == 0), stop=(i == nkt - 1),
                        skip_group_check=True,
                    )

            # stagger: emit scores+exp for a group of 4 t_k tiles, then the
            # av matmuls of the previous group, so PE never waits on exp.
            G = 4
            prev = []
            for g0 in range(0, nkt, G):
                cur = [(i, do_scores(i)) for i in range(g0, min(g0 + G, nkt))]
                for i, pts in prev:
                    do_avs(i, pts)
                prev = cur
            for i, pts in prev:
                do_avs(i, pts)

            # --- denominators, fully on-chip: reciprocal of the PSUM ones-row
            # then a K=1 ones matmul broadcasts it into partitions 64:128 of
            # the same PSUM bank; one DVE multiply normalizes.
            attT = attT_pool.tile([128, TC], mdt, name="attT")
            av_sbs = []
            for hh in range(2):
                av_sb = avs_pool.tile([HS + 1, TC], f32, name="av_sb")
                nc.vector.tensor_copy(av_sb, av_ps[hh][0:HS + 1, :])
                av_sbs.append(av_sb)
            # transpose the 8 denominator row-pieces into columns so ONE
            # 128-lane reciprocal covers the whole head-pair (a [1,512] row
            # reciprocal is single-lane and ~5 cycles/elem)
            rbt_ps = ps_mm.tile([TS, 2, NSUB], f32, name="rbt_ps", tag="mm")
            for hh in range(2):
                for j in range(NSUB):
                    nc.tensor.transpose(
                        rbt_ps[:, hh, j:j + 1],
                        av_sbs[hh][HS:HS + 1, j * TS:(j + 1) * TS],
                        ones65[HS:HS + 1, :],
                    )
            rec_cols = rec_pool.tile([TS, 2 * NSUB], mdt, name="rec_cols")
            with nc.allow_low_precision(reason="bf16 softmax denominators"):
                nc.vector.reciprocal(
                    out=rec_cols, in_=rbt_ps.rearrange("p a b -> p (a b)")
                )
            # transpose the reciprocal columns back into one [1, 1024] row
            # (partition 0, a different free offset per piece), copy it to
            # SBUF on the Act engine, and broadcast per piece via K=1 matmuls
            rr_ps = ps_mm.tile([1, 2, NSUB, TS], mdt, name="rr_ps", tag="mm")
            for hh in range(2):
                for j in range(NSUB):
                    nc.tensor.transpose(
                        rr_ps[0:1, hh, j, :],
                        rec_cols[:, hh * NSUB + j:hh * NSUB + j + 1], id_sb,
                    )
            rr_sb = rec_pool.tile([1, 2, NSUB, TS], mdt, name="rr_sb")
            nc.vector.tensor_copy(rr_sb, rr_ps)
            for hh in range(2):
                for j in range(NSUB):
                    nc.tensor.matmul(
                        av_ps[hh][HS:HS + HS, j * TS:(j + 1) * TS],
                        ones64b, rr_sb[0:1, hh, j, :],
                        start=True, stop=True, skip_group_check=True,
                    )
                nc.vector.tensor_mul(
                    attT[hh * HS:(hh + 1) * HS, :], av_sbs[hh][0:HS, :],
                    av_ps[hh][HS:HS + HS, :],
                )
            attTs.append(attT)
            if fillers:
                fillers.pop(0)()
        for f in fillers:
            f()
        return attTs

    def proj_subtile(c, attTs, s):
        # ar_in layout: region A rows [0,1024) = [chunk0; chunk2]; region B
        # rows [1024,2048) split into two sub-regions of [c1 pair; c3 pair]
        # so each (mini-)ReduceScatter input is contiguous.
        r0 = c * TC + s * TS
        if c % 2 == 0:
            w0 = (c // 2) * TC + s * TS
        else:
            w0 = 2 * TC + (s // 2) * TC + (c // 2) * 2 * TS + (s % 2) * TS
        xb_t = xb_pool.tile([128, E], mdt, name="xb_t")
        nc.sync.dma_start(out=xb_t, in_=xb[r0:r0 + TS, :])
        part = part_pool.tile([128, E], mdt, name="part")
        for n in range(2):
            ps = ps_mm.tile([128, TC], f32, name="ps_pr", tag="mm")
            for dd in range(NDT):
                nc.tensor.matmul(
                    ps, attTs[dd][:, s * TS:(s + 1) * TS],
                    wp_sb[:, dd, n * TC:(n + 1) * TC],
                    start=(dd == 0), stop=(dd == NDT - 1),
                )
            nc.vector.tensor_add(
                part[:, n * TC:(n + 1) * TC], ps, xb_t[:, n * TC:(n + 1) * TC]
            )
        nc.sync.dma_start(out=ar_in[w0:w0 + TS, :], in_=part)

    def proj_chunk(c, attTs):
        for s in range(NSUB):
            proj_subtile(c, attTs, s)

    def proj_fillers(c, attTs):
        from functools import partial
        return [partial(proj_subtile, c, attTs, s) for s in range(NSUB)]

    # =====================================================================
    # Phase 3: LN2 + FFN + residual for one reduced region (one chunk/core)
    # =====================================================================
    def ffn_half(r, src, half, h2T, f1):
        """LN2 + FFN for token subtiles {2*half, 2*half+1} of region r, so
        the second half of the region can wait on a later mini-collective
        while the first half computes."""
        c0 = half * 2 * TS
        x2_ts = []
        h2_ts = []
        for s in (2 * half, 2 * half + 1):
            x2_t = x2_pool.tile([128, E], mdt, name="x2_t")
            # SP DMA queue: its ReduceScatter wait must not block the gpsimd
            # queue (proj loads/stores) or the Act queue (the exp stream)
            nc.sync.dma_start(out=x2_t, in_=src(s))
            x2_ts.append(x2_t)
            h2_ts.append(h_pool.tile([128, E], mdt, name="h2_t", tag="h_t"))
        layer_norm_chunk(x2_ts, eps2_sb, h2_ts)
        transpose_cast(h2_ts, ln_sb["ln2g"], ln_sb["ln2b"], h2T, col0=c0)
        ps_f = ps_mm.tile([FFN, 2 * TS], f32, name="ps_f", tag="mm")
        for k in range(NET):
            nc.tensor.matmul(
                ps_f, w1_sb[:, k, :], h2T[:, k, c0:c0 + 2 * TS],
                start=(k == 0), stop=(k == NET - 1),
            )
        nc.scalar.activation(
            out=f1[0:FFN, c0:c0 + 2 * TS], in_=ps_f, func=AF.Relu,
            bias=b1_sb, scale=1.0,
        )
        for i, s in enumerate((2 * half, 2 * half + 1)):
            o_t = out_pool.tile([128, E], f32, name="o_t")
            for n in range(2):
                ps = ps_mm.tile([128, TC], f32, name="ps_o", tag="mm")
                nc.tensor.matmul(
                    ps, f1[:, s * TS:(s + 1) * TS],
                    w2_sb[:, n * TC:(n + 1) * TC],
                    start=True, stop=True,
                )
                nc.vector.tensor_add(
                    o_t[:, n * TC:(n + 1) * TC], ps,
                    x2_ts[i][:, n * TC:(n + 1) * TC],
                )
            r0 = r * TC + s * TS
            nc.scalar.dma_start(out=out[r0:r0 + TS, :], in_=o_t)

    def ffn_prep(r):
        h2T = hT_pool.tile([ET, NET, TC], mdt, name="h2T")
        f1 = f1_pool.tile([FFN + 1, TC], mdt, name="f1")
        nc.vector.memset(f1, 1.0)  # row FFN stays 1.0 (b2 matmul row)
        return h2T, f1

    def ffn_region(r, src):
        h2T, f1 = ffn_prep(r)
        ffn_half(r, src, 0, h2T, f1)
        ffn_half(r, src, 1, h2T, f1)

    # ---- schedule: att(0)/att(2) pulled into the QKV phase; proj subtiles
    # of completed chunks (and ffn_region(0)) interleave into later attention
    # chunks' head-pair loops as Tensor-engine filler ----
    ln_qkv_chunk(0, after_loads=load_qkv_weights)
    load_late_weights()
    ln_qkv_chunk(1)
    ln_qkv_chunk(2)
    attTs0 = attention_chunk(0)
    ln_qkv_chunk(3)
    attTs2 = attention_chunk(2, proj_fillers(0, attTs0))

    def rs_a():
        nc.gpsimd.collective_compute(
            "ReduceScatter", mybir.AluOpType.add, replica_groups=PAIRS,
            ins=[ar_in[0:2 * TC, :]], outs=[rs_out0],
        )

    # pack proj(2) into the first two head-pair slots of att(1) and launch
    # the first ReduceScatter right after, so it overlaps most of att(1)+att(3)
    p2 = proj_fillers(2, attTs2)
    attTs1 = attention_chunk(1, [
        lambda: (p2[0](), p2[1]()),
        lambda: (p2[2](), p2[3](), rs_a()),
    ])
    attTs3 = attention_chunk(
        3, proj_fillers(1, attTs1)
        + [lambda: ffn_region(0, lambda s: rs_out0[s * TS:(s + 1) * TS, :])]
    )
    proj_subtile(3, attTs3, 0)
    proj_subtile(3, attTs3, 1)
    nc.gpsimd.collective_compute(
        "ReduceScatter", mybir.AluOpType.add, replica_groups=PAIRS,
        ins=[ar_in[2 * TC:3 * TC, :]], outs=[rs_outB[0]],
    )
    proj_subtile(3, attTs3, 2)
    proj_subtile(3, attTs3, 3)
    nc.gpsimd.collective_compute(
        "ReduceScatter", mybir.AluOpType.add, replica_groups=PAIRS,
        ins=[ar_in[3 * TC:4 * TC, :]], outs=[rs_outB[1]],
    )
    srcB = lambda s: rs_outB[s // 2][(s % 2) * TS:(s % 2 + 1) * TS, :]
    h2T1, f11 = ffn_prep(1)
    ffn_half(1, srcB, 0, h2T1, f11)   # gated by mini-RS B1: overlaps B2
    ffn_half(1, srcB, 1, h2T1, f11)


# =========================================================================
# Host side
# =========================================================================
_NC_CACHE = {}
RUN_KWARGS = {}      # test harness may set {"trace": True} for profiling
LAST_RESULT = None   # BassKernelResults of the most recent run


def kernel(x, wq, wk, wv, w_proj, b_proj, w1, b1, w2, b2, ln1_g, ln1_b, ln2_g,
           ln2_b):
    mode = MM_MODE
    np_mdt = _np_mdt(mode)
    if mode not in _NC_CACHE:
        _NC_CACHE[mode] = build(mode)
    nc = _NC_CACHE[mode]

    x = np.asarray(x, np.float32)
    # lower-triangle [128,128] block mask: valid (1.0) iff t_k(p) <= t_q(f)
    mask = np.tile(
        np.tril(np.ones((TS, TS), np.float32)).T[:, None, :], (1, 2, 1)
    ).astype(np_mdt)
    identity = np.eye(TS, dtype=np.float32).astype(np_mdt)
    w2e = np.concatenate([np.asarray(w2, np.float32),
                          np.asarray(b2, np.float32)[None, :]], axis=0)

    def prearr(w, p):
        # [K, D] -> [p, K//p, D]: partition-major so the DMA is contiguous
        w = np.ascontiguousarray(w)
        return np.ascontiguousarray(
            w.reshape(w.shape[0] // p, p, w.shape[1]).transpose(1, 0, 2)
        )
    in_maps = []
    for core in range(NCORE):
        b, g = core // 2, core % 2
        sl = slice(g * DSL, (g + 1) * DSL)
        in_maps.append({
            "xr": (0.5 * x[b]).astype(np_mdt),
            "xb": (0.5 * x[b] + 0.5 * np.asarray(b_proj, np.float32)[None, :]).astype(np_mdt),
            "wq": prearr(np.asarray(wq, np.float32)[:, sl].astype(np_mdt), ET),
            "wk": prearr(np.asarray(wk, np.float32)[:, sl].astype(np_mdt), ET),
            "wv": prearr(np.asarray(wv, np.float32)[:, sl].astype(np_mdt), ET),
            "wp": prearr(
                np.asarray(w_proj, np.float32)[sl, :].astype(np_mdt), 128
            ),
            "w1": prearr(np.asarray(w1, np.float32).astype(np_mdt), ET),
            "w2e": w2e.astype(np_mdt),
            "b1": np.asarray(b1, np.float32)[:, None],
            "ln1g": np.asarray(ln1_g, np.float32)[:, None],
            "ln1b": np.asarray(ln1_b, np.float32)[:, None],
            "ln2g": np.asarray(ln2_g, np.float32)[:, None],
            "ln2b": np.asarray(ln2_b, np.float32)[:, None],
            "mask": mask,
            "ident": identity,
        })
    global LAST_RESULT
    res = run_bass_kernel_spmd(nc, in_maps, list(range(NCORE)), **RUN_KWARGS)
    LAST_RESULT = res
    outp = np.empty((B, T, E), np.float32)
    for core in range(NCORE):
        b, g = core // 2, core % 2
        outp[b, g * (T // 2):(g + 1) * (T // 2), :] = res.results[core]["out"]
    return outp
